# revision 20
# baseline (speedup 1.0000x reference)
"""TRN2 Bass kernel for nn_Attention_75935021793702.

Dense transformer attention block:
    qkv = x @ Wqkv ; q,k = RoPE(q,k,pos) ; y = softmax(causal(q k^T / sqrt(dk))) v ; out = y @ Wo

Sharding: 8-way tensor-parallel over heads (2 heads/core).  Each core computes
its heads' qkv projection (column slice of Wqkv), attention for its (B, head)
pairs, and a partial output projection (row slice of Wo).  The host sums the 8
partial outputs.

All matmul operands are float16 (11-bit significand, ~ fp32r precision, but
half the SBUF/DMA footprint and full PE rate at any moving width).

Device dataflow (per core):
  Phase A: stream x^T -> q^T,k^T (feature-major, RoPE on DVE) and v
           (token-major); q/k/v stay RESIDENT in SBUF (no DRAM spill).
  Phase B (fused attention + output projection), per (batch, tq-chunk, head):
    per tk-tile: S^T = k^T^T q^T on PE; exp on ACT (scale=1/sqrt(dk)) -> es
    (fp16); causal mask applied post-exp as a [128,128] 0/1 window multiply
    on GPSIMD (keeps DVE/ACT out of the PSUM-recycle path); A@V and row-sum
    (ones-matmul) accumulate on PE; normalize O^T by 1/rowsum
    (approx-reciprocal + DRAM-broadcast DMA + DVE mul).
    All deferred work (trailing A@V, reciprocal+broadcast, normalize) and
    the output projection of older chunks is drained one unit per S-matmul
    inside later attention instances, so PE never waits on ACT/DVE/DMA
    latency and the projection's PSUM->SBUF copies never pile up in the
    DVE/ACT queues ahead of the mask/exp chain.
"""

import sys

sys.path.insert(0, "/opt/trn_rl_repo")

import numpy as np
import concourse.bass as bass
import concourse.mybir as mybir
import concourse.tile as tile
from concourse import bacc
from concourse.bass_utils import run_bass_kernel_spmd

F32 = mybir.dt.float32
F16 = mybir.dt.float16
EXP = mybir.ActivationFunctionType.Exp

MM_DT = F16           # dtype of all matmul operands
NP_MM = np.float16

B, T, D, H = 2, 2048, 2048, 16
DK = D // H                       # 128
THETA = 10000.0
NCORES = 8
HPC = H // NCORES                 # heads per core = 2
BT = B * T                        # 4096
DL = HPC * DK                     # local d width = 256
TCH = 512                         # token chunk (matmul moving dim)
NCH = BT // TCH                   # 8 chunks over both batches
NCHB = T // TCH                   # 4 chunks per batch
KT = D // 128                     # 16 contraction tiles
NTT = T // 128                    # tk tiles per batch = 16
SCALE = 1.0 / float(np.sqrt(np.float32(DK)))

_cache = {}


def _mm(nc, out, lhsT, rhs, start, stop):
    nc.tensor.matmul(out, lhsT, rhs, start=start, stop=stop)


def _build(tile_kinds, n_pat):
    """tile_kinds[j][i] for tq-chunk j, tk-tile i (within one batch):
    ('full',) | ('skip',) | ('part', pat_idx, sl, w0) where sl is the
    128-aligned first computed column and [w0, w0+128) the mask window."""
    nc = bacc.Bacc("TRN2", target_bir_lowering=False, debug=False)

    xt_d = nc.dram_tensor("xt", [D, BT], MM_DT, kind="ExternalInput").ap()
    wqkv_d = nc.dram_tensor("wqkv", [D, 3 * DL], MM_DT, kind="ExternalInput").ap()
    wo_d = nc.dram_tensor("wo", [DL, D], MM_DT, kind="ExternalInput").ap()
    cs2_d = nc.dram_tensor("cs2", [DK, BT], F16, kind="ExternalInput").ap()
    sn2_d = nc.dram_tensor("sn2", [DK, BT], F16, kind="ExternalInput").ap()
    mt_d = nc.dram_tensor("mt", [max(n_pat, 1), 128, 128], F16, kind="ExternalInput").ap()
    # output is stored transposed [D, BT]; the host transposes back.  This
    # lets the projection run with the resident Wo slices as the stationary
    # operand (LDWEIGHTS always ready) and O^T as the moving operand.
    out_d = nc.dram_tensor("out", [D, BT], MM_DT, kind="ExternalOutput").ap()

    with tile.TileContext(nc) as tc:
        with tc.tile_pool(name="dram", bufs=1, space="DRAM") as dp, \
             tc.tile_pool(name="const", bufs=1) as pc, \
             tc.tile_pool(name="res", bufs=1) as pr:
            r_sp = dp.tile([B * HPC * NCHB, TCH], F32)  # 1/rowsum rows
            # resident q/k/v for the whole problem (both batches)
            qk_res = pr.tile([128, 4, BT], MM_DT)   # nt 0,1: qT h0,h1; 2,3: kT h0,h1
            v_res = pr.tile([128, NTT * B, DL], MM_DT)  # token-major v tiles

            # ---------------- Phase A: QKV + RoPE ----------------
            with (
                tc.tile_pool(name="pa", bufs=1) as pa,
                tc.tile_pool(name="pax", bufs=32) as pax,
                tc.tile_pool(name="pat", bufs=4) as pat,
                tc.tile_pool(name="pap", bufs=3, space="PSUM") as pap,
            ):
                wq = pa.tile([128, KT, 3 * DL], MM_DT)
                cs2 = pa.tile([128, BT], F16)
                sn2 = pa.tile([128, BT], F16)

                for tch in range(NCH):
                    tc0 = tch * TCH
                    xts = []
                    for ki in range(KT):
                        if tch == 0:
                            # interleave weight and first-chunk activation loads
                            nc.sync.dma_start(wq[:, ki, :], wqkv_d[128 * ki : 128 * ki + 128, :])
                        xt = pax.tile([128, TCH], MM_DT, tag="xt", name=f"xt_{tch}_{ki}")
                        nc.sync.dma_start(xt[:, :], xt_d[128 * ki : 128 * ki + 128, tc0 : tc0 + TCH])
                        xts.append(xt)
                    if tch == 0:
                        nc.sync.dma_start(cs2[:, 0:T], cs2_d[:, 0:T])
                        nc.sync.dma_start(sn2[:, 0:T], sn2_d[:, 0:T])
                    if tch == 1:
                        nc.sync.dma_start(cs2[:, T:BT], cs2_d[:, T:BT])
                        nc.sync.dma_start(sn2[:, T:BT], sn2_d[:, T:BT])
                    # q,k feature-major (4 head-tiles: q0,q1,k0,k1) + RoPE
                    for nt in range(4):
                        psqk = pap.tile([128, TCH], F32, tag="psqk", name=f"psqk_{tch}_{nt}")
                        for ki in range(KT):
                            _mm(nc, psqk[:, :], wq[:, ki, 128 * nt : 128 * nt + 128],
                                xts[ki][:, :], ki == 0, ki == KT - 1)
                        t2 = pat.tile([128, TCH], F32, tag="t2", name=f"t2_{tch}_{nt}")
                        nc.vector.tensor_mul(t2[0:64, :], psqk[64:128, :], sn2[0:64, tc0 : tc0 + TCH])
                        nc.vector.tensor_mul(t2[64:128, :], psqk[0:64, :], sn2[64:128, tc0 : tc0 + TCH])
                        t1 = pat.tile([128, TCH], F32, tag="t1", name=f"t1_{tch}_{nt}")
                        nc.vector.tensor_mul(t1[:, :], psqk[:, :], cs2[:, tc0 : tc0 + TCH])
                        nc.vector.tensor_add(qk_res[:, nt, tc0 : tc0 + TCH], t1[:, :], t2[:, :])
                    # v token-major, straight into the resident tile
                    for tt in range(4):
                        psv = pap.tile([128, DL], F32, tag="psv", name=f"psv_{tch}_{tt}")
                        for ki in range(KT):
                            _mm(nc, psv[:, :], xts[ki][:, 128 * tt : 128 * tt + 128],
                                wq[:, ki, 2 * DL : 3 * DL], ki == 0, ki == KT - 1)
                        nc.vector.tensor_copy(v_res[:, 4 * tch + tt, :], psv[:, :])
                    if tch == 2:
                        # constants for phase B, loaded mid-phase-A
                        wo = pc.tile([128, HPC, D], MM_DT)
                        for dt in range(HPC):
                            nc.sync.dma_start(wo[:, dt, :], wo_d[128 * dt : 128 * dt + 128, :])
                        mts = pc.tile([128, max(n_pat, 1), 128], F16)
                        for pi in range(n_pat):
                            nc.sync.dma_start(mts[:, pi, :], mt_d[pi, :, :])
                        ones_f32 = pc.tile([128, 1], F32)
                        nc.vector.memset(ones_f32[:, :], 1.0)
                        ones = pc.tile([128, 1], MM_DT)
                        nc.vector.tensor_copy(ones[:, :], ones_f32[:, :])

            # ------- Phase B: attention + output projection (fused) -------
            with (
                tc.tile_pool(name="pbe", bufs=8) as pbe,
                tc.tile_pool(name="pbo", bufs=3) as pbo,
                tc.tile_pool(name="pbz", bufs=6) as pbz,
                tc.tile_pool(name="pcs", bufs=3) as pcs,
                tc.tile_pool(name="pbs", bufs=2, space="PSUM") as pbs,
                tc.tile_pool(name="pbp", bufs=2, space="PSUM") as pbp,
                tc.tile_pool(name="pcp", bufs=2, space="PSUM") as pcp,
            ):
                # deferred work: tail (trailing A@V+rowsum, then recip+DMAs)
                # and fin (normalize) of the previous attention instance;
                # pwork = projection units of older chunks.  One unit is
                # drained per S-matmul inside attention instances.
                pending = {"tail": [], "fin": None, "pwork": []}

                def drain_slot():
                    if pending["tail"]:
                        pending["tail"].pop(0)()
                    elif pending["fin"] is not None:
                        fn = pending["fin"]
                        pending["fin"] = None
                        fn()
                    elif pending["pwork"]:
                        pending["pwork"].pop(0)()
                        if len(pending["pwork"]) > 8:
                            pending["pwork"].pop(0)()

                def flush_tail_fin():
                    while pending["tail"]:
                        pending["tail"].pop(0)()
                    if pending["fin"] is not None:
                        fn = pending["fin"]
                        pending["fin"] = None
                        fn()

                osbs = {}

                def attention(b, j, h):
                    col0 = b * T
                    qc0 = col0 + TCH * j
                    rrow = (b * HPC + h) * NCHB + j
                    qt = qk_res[:, h, qc0 : qc0 + TCH]
                    kinds = tile_kinds[j]
                    live = [i for i in range(NTT) if kinds[i][0] != "skip"]
                    ps_o = pbp.tile([128, TCH], F32, tag="ps_o", name=f"pso_{b}_{h}_{j}")
                    ps_r = pbp.tile([1, TCH], F32, tag="ps_r", name=f"psr_{b}_{h}_{j}")
                    ess = {}
                    SKEW = 2

                    def consume(ii, idx):
                        st = idx == 0
                        sp = idx == len(live) - 1
                        es, sl = ess[ii]
                        _mm(nc, ps_o[:, sl:TCH],
                            v_res[:, b * NTT + ii, DK * h : DK * h + DK],
                            es[:, sl:TCH], st, sp)
                        _mm(nc, ps_r[:, sl:TCH], ones[:, :], es[:, sl:TCH], st, sp)

                    for idx, i in enumerate(live):
                        kind = kinds[i]
                        sl = kind[2] if kind[0] == "part" else 0
                        ps_s = pbs.tile([128, TCH], F32, tag="ps_s", name=f"pss_{b}_{h}_{j}_{i}")
                        _mm(nc, ps_s[:, sl:TCH],
                            qk_res[:, 2 + h, col0 + 128 * i : col0 + 128 * i + 128],
                            qt[:, sl:TCH], True, True)
                        es = pbe.tile([128, TCH], MM_DT, tag="es", name=f"es_{b}_{h}_{j}_{i}")
                        nc.scalar.activation(es[:, sl:TCH], ps_s[:, sl:TCH], EXP, scale=SCALE)
                        if kind[0] == "part" and kind[1] >= 0:
                            # zero the masked triangle post-exp on the (idle)
                            # GPSIMD engine, keeping DVE/ACT out of the ps_s
                            # recycle path
                            _, pi, _, w0 = kind
                            nc.gpsimd.tensor_mul(es[:, w0 : w0 + 128], es[:, w0 : w0 + 128],
                                                 mts[:, pi, :])
                        ess[i] = (es, sl)
                        drain_slot()
                        if idx >= SKEW:
                            consume(live[idx - SKEW], idx - SKEW)
                    flush_tail_fin()

                    def recip_dma():
                        rs = pbo.tile([1, TCH], F32, tag="rs", name=f"rs_{b}_{h}_{j}")
                        nc.vector.reciprocal_approx_fast(rs[:, :], ps_r[0:1, :])
                        nc.sync.dma_start(r_sp[rrow : rrow + 1, :], rs[:, :])
                        rbc = pbo.tile([128, TCH], F32, tag="rbc", name=f"rbc_{b}_{h}_{j}")
                        nc.sync.dma_start(
                            rbc[:, :],
                            r_sp[rrow : rrow + 1, :].to_broadcast((128, TCH)),
                        )
                        pending[("rbc", b, h, j)] = rbc

                    def finalize():
                        rbc = pending.pop(("rbc", b, h, j))
                        osb = pbz.tile([128, TCH], MM_DT, tag="osb", name=f"osb_{b}_{h}_{j}")
                        nc.vector.tensor_mul(osb[:, :], ps_o[:, :], rbc[:, :])
                        osbs[(b, h, j)] = osb

                    pending["tail"] = [
                        (lambda idx=idx: consume(live[idx], idx))
                        for idx in range(max(0, len(live) - SKEW), len(live))
                    ] + [recip_dma]
                    pending["fin"] = finalize

                def project(b, j, final=False):
                    # out^T[dcol, tq] += Wo[dl, dcol]^T O^T[dl, tq] for chunk
                    # (b, j), split into 16 units drained inside later
                    # instances.  Stationary = Wo slice (resident constant).
                    # pso uses a single PSUM bank: units are spread one per
                    # attention slot, so the PSUM->SBUF copy overlaps other PE
                    # work.  The final bunched flush instead borrows the ps_o
                    # banks, idle once attention is done.
                    col0 = b * T
                    # force any older projection units out first
                    while pending["pwork"]:
                        pending["pwork"].pop(0)()
                    units = []
                    for dc in range(D // 128):
                        def unit(dc=dc, b=b, j=j, col0=col0):
                            tcol = col0 + TCH * j
                            if final and dc % 2:
                                pso = pbp.tile([128, TCH], F32, tag="ps_o", name=f"psoc_{b}_{j}_{dc}")
                            else:
                                pso = pcp.tile([128, TCH], F32, tag="pso", name=f"psoc_{b}_{j}_{dc}")
                            for hh in range(HPC):
                                _mm(nc, pso[:, :],
                                    wo[:, hh, 128 * dc : 128 * dc + 128],
                                    osbs[(b, hh, j)][:, :],
                                    hh == 0, hh == HPC - 1)
                            outsb = pcs.tile([128, TCH], MM_DT, tag="outsb", name=f"outsb_{b}_{j}_{dc}")
                            nc.vector.tensor_copy(outsb[:, :], pso[:, :])
                            nc.sync.dma_start(
                                out_d[128 * dc : 128 * dc + 128, tcol : tcol + TCH],
                                outsb[:, :],
                            )
                        units.append(unit)
                    pending["pwork"] = units

                chunks = [(b, j) for b in range(B) for j in range(NCHB)]
                for ci, (b, j) in enumerate(chunks):
                    if ci > 0:
                        project(*chunks[ci - 1])
                    attention(b, j, 0)
                    attention(b, j, 1)
                flush_tail_fin()
                while pending["pwork"]:
                    pending["pwork"].pop(0)()
                project(*chunks[-1], final=True)
                while pending["pwork"]:
                    pending["pwork"].pop(0)()

    nc.compile()
    return nc


def _mask_tiles(mask):
    """Classify causal-mask tiles (within one batch).  Returns (tile_kinds,
    0/1 keep-mask window patterns [tk=128, u=128], fp16)."""
    m = np.asarray(mask[0, 0])
    pats = []
    pat_idx = {}
    tile_kinds = []
    for j in range(NCHB):
        row = []
        for i in range(T // 128):
            blk = m[TCH * j : TCH * j + TCH, 128 * i : 128 * i + 128]  # [tq, tk]
            if blk.all():
                row.append(("full",))
                continue
            if not blk.any():
                row.append(("skip",))
                continue
            valid_rows = np.nonzero(blk.any(axis=1))[0]
            first_valid = int(valid_rows[0])
            sl = (first_valid // 128) * 128
            mb = ~blk  # masked positions [tq, tk]
            mrows = np.nonzero(mb[sl:, :].any(axis=1))[0]
            if mrows.size == 0:
                # fully valid beyond sl; no mask add needed
                row.append(("part", -1, sl, 0))
                continue
            w0 = sl + int(mrows[0])
            w1 = sl + int(mrows[-1]) + 1
            assert w1 - w0 <= 128, "mask window wider than 128 not supported"
            w0 = min(w0, TCH - 128)
            sub = mb[w0 : w0 + 128, :]  # [u, tk]
            P = np.ones((128, 128), np.float16)
            P[:, : sub.shape[0]] = 1.0 - sub.T.astype(np.float16)
            key = P.tobytes()
            if key not in pat_idx:
                pat_idx[key] = len(pats)
                pats.append(P)
            row.append(("part", pat_idx[key], sl, w0))
        # first live tile must cover the full chunk width (ps_o start clears)
        live = [k for k in row if k[0] != "skip"]
        assert live and (live[0][0] == "full" or live[0][2] == 0)
        tile_kinds.append(row)
    return tile_kinds, pats


def _prep_inputs(x, mask, pos, Wqkv, Wo):
    xT = np.ascontiguousarray(
        np.asarray(x, dtype=np.float32).reshape(BT, D).T
    ).astype(NP_MM)
    pos = np.asarray(pos)
    inv = (
        np.float32(1.0)
        / (np.float32(THETA) ** (np.arange(0, DK, 2, dtype=np.float32) / np.float32(DK)))
    ).astype(np.float32)
    ang = pos.astype(np.float32)[:, None] * inv[None, :]  # [T, 64]
    cosT = np.cos(ang).astype(np.float32).T  # [64, T]
    sinT = np.sin(ang).astype(np.float32).T
    cs2 = np.tile(np.concatenate([cosT, cosT], 0), (1, B))  # [128, BT]
    sn2 = np.tile(np.concatenate([-sinT, sinT], 0), (1, B))
    cs2 = np.ascontiguousarray(cs2, dtype=np.float16)
    sn2 = np.ascontiguousarray(sn2, dtype=np.float16)

    tile_kinds, pats = _mask_tiles(mask)
    n_pat = len(pats)
    mt = (
        np.stack(pats, 0)
        if n_pat
        else np.zeros((1, 128, 128), dtype=np.float16)
    ).astype(np.float16)

    Wqkv = np.asarray(Wqkv, dtype=np.float32)
    Wo = np.asarray(Wo, dtype=np.float32)
    in_maps = []
    for g in range(NCORES):
        c0 = g * DL
        wqkv_g = np.ascontiguousarray(
            np.concatenate(
                [Wqkv[:, c0 : c0 + DL], Wqkv[:, D + c0 : D + c0 + DL],
                 Wqkv[:, 2 * D + c0 : 2 * D + c0 + DL]], axis=1)
        ).astype(NP_MM)
        wo_g = np.ascontiguousarray(Wo[c0 : c0 + DL, :]).astype(NP_MM)
        in_maps.append(
            {"xt": xT, "wqkv": wqkv_g, "wo": wo_g, "cs2": cs2, "sn2": sn2, "mt": mt}
        )
    return in_maps, tile_kinds, n_pat


def _get_nc(tile_kinds, n_pat):
    key = (str(tile_kinds), n_pat)
    if key not in _cache:
        _cache[key] = _build(tile_kinds, n_pat)
    return _cache[key]


def run(x, mask, pos, Wqkv, Wo, trace=False):
    in_maps, tile_kinds, n_pat = _prep_inputs(x, mask, pos, Wqkv, Wo)
    nc = _get_nc(tile_kinds, n_pat)
    res = run_bass_kernel_spmd(nc, in_maps, core_ids=list(range(NCORES)), trace=trace)
    total = np.zeros((BT, D), dtype=np.float64)
    for r in res.results:
        total += r["out"].astype(np.float64).T
    out = total.astype(np.float32).reshape(B, T, D)
    return out, res


def kernel(x, mask, pos, Wqkv, Wo):
    out, _ = run(x, mask, pos, Wqkv, Wo, trace=False)
    return out
